# revision 18
# baseline (speedup 1.0000x reference)
"""Trainium2 Bass kernel for Llama3-style GQA attention with streaming KV eviction.

Sharding: tensor-parallel over heads across 8 NeuronCores. Each core owns 4
query heads and their single shared KV head (one full GQA group). Wq/Wk/Wv are
sharded on the output (head) dim, Wo on the input dim; the 8 per-core Wo
partials are summed on the host. Sink+recent KV eviction is head-local.

Per-core dataflow (all matmul operands fp16, fp32 PSUM accumulation):
  1. Token-major QKV projection streaming hidden.T in 512-token chunks.
  2. RoPE in token-major layout (rotate-half is a free-dim swap done with a
     single negative-step access pattern), 1/sqrt(D) folded into the exp scale.
  3. PE transposes build d-major qT/kT for attention; v stays token-major.
  4. Scores computed transposed (ST[kt, qt] = kT_blk.T @ qT) so exp(ST) is
     directly the rhs of the P@V matmul - no P transpose, no running max
     (scores are bounded, fp32 exp is safe unnormalized).
  5. Row sums via ones-matmul on PE into one PSUM bank (head h at partition
     32h), reciprocal on DVE, GPSIMD partition-broadcast, normalize into fp16.
  6. Wo matmul from d-major attnT; fp32 partial out DMA'd per core.
"""

import numpy as np

B, S, HID = 1, 2048, 4096
H, KVH, D = 32, 8, 128
THETA = 500000.0
SINK = 4
RECENT = 204  # int(S * 0.1)
NREC = SINK + RECENT  # 208

NCORES = 8
HPC = H // NCORES          # 4 q heads per core
DOUT = HPC * D             # 512
KC = HID // 128            # 32 contraction chunks
TB = S // 128              # 16 token blocks
QC = S // 512              # 4 query chunks
SCALE = float(1.0 / np.sqrt(D))

_CACHE = {}


def _swapped_halves(ap, nchunks):
    """View of ap's first nchunks*128 cols with 64-wide halves swapped per
    128-chunk: out[:, c*128 + j] = ap[:, c*128 + 64 + j] (j<64), ap[:, c*128 + j - 64] (j>=64)."""
    import concourse.bass as bass
    return bass.AP(
        tensor=ap.tensor,
        offset=ap.offset + 64,
        ap=[ap.ap[0], [128, nchunks], [-64, 2], [1, 64]],
    )


def _build_nc():
    import concourse.bass as bass
    import concourse.tile as tile
    from concourse import bacc, mybir
    from concourse.masks import make_identity

    F16 = mybir.dt.float16
    F32 = mybir.dt.float32
    EXP = mybir.ActivationFunctionType.Exp

    nc = bacc.Bacc("TRN2", debug=False)

    hT = nc.dram_tensor("hT", [HID, S], F16, kind="ExternalInput").ap()
    wall = nc.dram_tensor("wall", [HID, DOUT + 256], F16, kind="ExternalInput").ap()
    woT = nc.dram_tensor("woT", [DOUT, HID], F16, kind="ExternalInput").ap()
    cos5 = nc.dram_tensor("cos5", [S, 640], F16, kind="ExternalInput").ap()
    sin5 = nc.dram_tensor("sin5", [S, 640], F16, kind="ExternalInput").ap()
    dmask = nc.dram_tensor("dmask", [128, 128], F16, kind="ExternalInput").ap()
    invd = nc.dram_tensor("invd", [QC, HPC, 512], F32, kind="Internal").ap()
    outp = nc.dram_tensor("outp", [S, HID], F32, kind="ExternalOutput").ap()
    krec = nc.dram_tensor("krec", [NREC, D], F32, kind="ExternalOutput").ap()
    vrec = nc.dram_tensor("vrec", [NREC, D], F32, kind="ExternalOutput").ap()

    with tile.TileContext(nc) as tc:
        from contextlib import ExitStack

        with ExitStack() as ctx:
            res = ctx.enter_context(tc.tile_pool(name="res", bufs=1))
            qT_sb = res.tile([128, HPC * S], F16)   # [d, h*S + t]
            kT_sb = res.tile([128, S], F16)         # [d, t]
            v_sb = res.tile([128, S], F16)          # [t%128, blk*128 + d]
            attnT = res.tile([128, HPC * S], F16)   # [d, h*S + t]
            ones32 = res.tile([128, 32], F16)
            nc.vector.memset(ones32, 1.0)
            ident = res.tile([128, 128], F16)
            make_identity(nc, ident)
            dmask_sb = res.tile([128, 128], F16)
            nc.sync.dma_start(out=dmask_sb, in_=dmask)
            # wait-carrier: TensorTensor's ISA struct has one sync-wait slot,
            # so pre-consume DMA'd tiles on DVE with tiny copies; later DVE
            # ops then inherit the dep transitively (add_sem_waits elides).
            dm_c = res.tile([1, 2], F16)
            nc.vector.tensor_copy(dm_c, dmask_sb[0:1, 0:2])

            # ---------------- Phase 1: projections + rope + transposes ----
            with ExitStack() as p1:
                wp = p1.enter_context(tc.tile_pool(name="wp", bufs=1))
                ph = p1.enter_context(tc.tile_pool(name="ph", bufs=2))
                trig = p1.enter_context(tc.tile_pool(name="trig", bufs=2))
                rp = p1.enter_context(tc.tile_pool(name="rp", bufs=3))
                pqp = p1.enter_context(tc.tile_pool(name="pqp", bufs=2, space="PSUM"))
                ptr = p1.enter_context(tc.tile_pool(name="ptr", bufs=3, space="PSUM"))

                wall_sb = wp.tile([128, KC, DOUT + 256], F16)
                nc.sync.dma_start(
                    out=wall_sb, in_=wall.rearrange("(kc p) d -> p kc d", p=128)
                )
                hT3 = hT.rearrange("(kc p) t -> p kc t", p=128)
                cos4 = cos5.rearrange("(n p) d -> p n d", p=128)
                sin4 = sin5.rearrange("(n p) d -> p n d", p=128)

                for tci in range(4):
                    hT_t = ph.tile([128, KC, 512], F16, tag="ht")
                    nc.sync.dma_start(out=hT_t, in_=hT3[:, :, tci * 512:(tci + 1) * 512])
                    cos_t = trig.tile([128, 4, 640], F16, tag="cos")
                    nc.sync.dma_start(out=cos_t, in_=cos4[:, tci * 4:(tci + 1) * 4, :])
                    sin_t = trig.tile([128, 4, 640], F16, tag="sin")
                    nc.sync.dma_start(out=sin_t, in_=sin4[:, tci * 4:(tci + 1) * 4, :])
                    tr_c = trig.tile([1, 4], F16, tag="trc")
                    nc.vector.tensor_copy(tr_c[0:1, 0:2], cos_t[0:1, 0, 0:2])
                    nc.vector.tensor_copy(tr_c[0:1, 2:4], sin_t[0:1, 0, 0:2])

                    for tbl in range(4):
                        tb = tci * 4 + tbl
                        pq = pqp.tile([128, DOUT + 256], F32, tag="pq")
                        for kc in range(KC):
                            lhs = hT_t[:, kc, tbl * 128:(tbl + 1) * 128]
                            nc.tensor.matmul(
                                pq[:, 0:512], lhs, wall_sb[:, kc, 0:512],
                                start=(kc == 0), stop=(kc == KC - 1),
                            )
                            nc.tensor.matmul(
                                pq[:, 512:768], lhs, wall_sb[:, kc, 512:768],
                                start=(kc == 0), stop=(kc == KC - 1),
                            )
                        # rope: qk' = qk*cos + swap(qk)*sin_signed
                        rot = rp.tile([128, 640], F16, tag="rot")
                        nc.vector.tensor_copy(rot, _swapped_halves(pq, 5))
                        tmp = rp.tile([128, 640], F16, tag="tmp")
                        nc.vector.tensor_mul(tmp, pq[:, 0:640], cos_t[:, tbl, :])
                        nc.vector.tensor_mul(rot, rot, sin_t[:, tbl, :])
                        qkr = rp.tile([128, 640], F16, tag="qkr")
                        nc.vector.tensor_add(qkr, tmp, rot)
                        # v: straight fp16 copy (token-major)
                        nc.vector.tensor_copy(
                            v_sb[:, tb * 128:(tb + 1) * 128], pq[:, 640:768]
                        )
                        # recents (k roped fp16->fp32 cast; v exact from psum):
                        # cast full aligned tiles, DMA arbitrary row slices.
                        if tb in (0, 14, 15):
                            k32 = rp.tile([128, D], F32, tag="k32")
                            v32 = rp.tile([128, D], F32, tag="v32")
                            nc.vector.tensor_copy(k32, qkr[:, 512:640])
                            nc.vector.tensor_copy(v32, pq[:, 640:768])
                            if tb == 0:
                                nc.sync.dma_start(out=krec[0:4, :], in_=k32[0:4, :])
                                nc.sync.dma_start(out=vrec[0:4, :], in_=v32[0:4, :])
                            elif tb == 14:
                                nc.sync.dma_start(out=krec[4:80, :], in_=k32[52:128, :])
                                nc.sync.dma_start(out=vrec[4:80, :], in_=v32[52:128, :])
                            else:
                                nc.sync.dma_start(out=krec[80:NREC, :], in_=k32)
                                nc.sync.dma_start(out=vrec[80:NREC, :], in_=v32)
                        # transposes to d-major
                        for c in range(5):
                            pt = ptr.tile([128, 128], F16, tag="tr")
                            nc.tensor.transpose(pt, qkr[:, c * 128:(c + 1) * 128], ident)
                            if c < HPC:
                                dst = qT_sb[:, c * S + tb * 128: c * S + (tb + 1) * 128]
                            else:
                                dst = kT_sb[:, tb * 128:(tb + 1) * 128]
                            nc.scalar.copy(dst, pt)

            # ---------------- Phase 2: attention -------------------------
            with ExitStack() as p2:
                ptp = p2.enter_context(tc.tile_pool(name="ptp", bufs=6))
                invp = p2.enter_context(tc.tile_pool(name="invp", bufs=2))
                bbp = p2.enter_context(tc.tile_pool(name="bbp", bufs=3))
                pA = p2.enter_context(tc.tile_pool(name="pA", bufs=4, space="PSUM"))
                pT = p2.enter_context(tc.tile_pool(name="pT", bufs=2, space="PSUM"))
                pS = p2.enter_context(tc.tile_pool(name="pS", bufs=1, space="PSUM"))

                for qc in range(QC):
                    nkb = 4 * (qc + 1)
                    As = [pA.tile([128, 512], F32, tag="A", name=f"A{h}") for h in range(HPC)]
                    s_all = pS.tile([128, 512], F32, tag="s")
                    for j in range(nkb):
                        m = j - 4 * qc
                        off = 128 * m if m >= 0 else 0
                        first = j == 0
                        last = j == nkb - 1
                        for h in range(HPC):
                            T = pT.tile([128, 512], F32, tag="T")
                            nc.tensor.matmul(
                                T[:, off:],
                                kT_sb[:, j * 128:(j + 1) * 128],
                                qT_sb[:, h * S + qc * 512 + off: h * S + (qc + 1) * 512],
                                start=True, stop=True,
                            )
                            P = ptp.tile([128, 512], F16, tag="P")
                            nc.scalar.activation(P[:, off:], T[:, off:], EXP, scale=SCALE)
                            if m >= 0:
                                nc.vector.tensor_mul(
                                    P[:, off:off + 128], P[:, off:off + 128], dmask_sb
                                )
                            nc.tensor.matmul(
                                As[h][:, off:],
                                v_sb[:, j * 128:(j + 1) * 128],
                                P[:, off:],
                                start=first, stop=last,
                            )
                            nc.tensor.matmul(
                                s_all[32 * h:32 * h + 32, off:],
                                ones32,
                                P[:, off:],
                                start=first, stop=last,
                                tile_position=(0, 32 * h),
                                # sim's psum group-check mis-addresses
                                # partition-offset outputs; data path is fine
                                skip_group_check=True,
                            )
                    inv = invp.tile([128, 512], F32, tag="inv")
                    nc.vector.reciprocal(inv, s_all)
                    # rows {0,32,64,96} hold the 4 heads' inverses; stage them
                    # in DRAM so a partition-step-0 DMA can broadcast them.
                    inv_rows = bass.AP(
                        tensor=inv.tensor, offset=inv.offset,
                        ap=[[32 * 512, HPC], [1, 512]],
                    )
                    nc.sync.dma_start(out=invd[qc], in_=inv_rows)
                    for h in range(HPC):
                        Bb = bbp.tile([128, 512], F32, tag="B")
                        bcast_src = bass.AP(
                            tensor=invd.tensor,
                            offset=(qc * HPC + h) * 512,
                            ap=[[0, 128], [1, 512]],
                        )
                        nc.sync.dma_start(out=Bb, in_=bcast_src)
                        b_c = bbp.tile([1, 2], F32, tag="bc")
                        nc.vector.tensor_copy(b_c, Bb[0:1, 0:2])
                        nc.vector.tensor_mul(
                            attnT[:, h * S + qc * 512: h * S + (qc + 1) * 512],
                            As[h], Bb,
                        )

            # ---------------- Phase 3: output projection -----------------
            with ExitStack() as p3:
                wop = p3.enter_context(tc.tile_pool(name="wop", bufs=1))
                osbp = p3.enter_context(tc.tile_pool(name="osbp", bufs=6))
                pop = p3.enter_context(tc.tile_pool(name="pop", bufs=8, space="PSUM"))

                wo_sb = wop.tile([128, HPC, HID], F16)
                nc.sync.dma_start(
                    out=wo_sb, in_=woT.rearrange("(h p) n -> p h n", p=128)
                )
                for tb in range(TB):
                    pos = [pop.tile([128, 512], F32, tag="po", name=f"po{i}") for i in range(8)]
                    for h in range(HPC):
                        lhs = attnT[:, h * S + tb * 128: h * S + (tb + 1) * 128]
                        for hc in range(8):
                            nc.tensor.matmul(
                                pos[hc], lhs, wo_sb[:, h, hc * 512:(hc + 1) * 512],
                                start=(h == 0), stop=(h == HPC - 1),
                            )
                    for hc in range(8):
                        ot = osbp.tile([128, 512], F32, tag="ot")
                        if hc % 2 == 0:
                            nc.scalar.copy(ot, pos[hc])
                        else:
                            nc.vector.tensor_copy(ot, pos[hc])
                        nc.sync.dma_start(
                            out=outp[tb * 128:(tb + 1) * 128, hc * 512:(hc + 1) * 512],
                            in_=ot,
                        )
    nc.compile()
    return nc


def get_nc():
    if "nc" not in _CACHE:
        _CACHE["nc"] = _build_nc()
    return _CACHE["nc"]


def make_in_maps(hidden_states, position_ids, Wq, Wk, Wv, Wo):
    hidden_states = np.asarray(hidden_states, dtype=np.float32)
    position_ids = np.asarray(position_ids)
    Wq = np.asarray(Wq, dtype=np.float32)
    Wk = np.asarray(Wk, dtype=np.float32)
    Wv = np.asarray(Wv, dtype=np.float32)
    Wo = np.asarray(Wo, dtype=np.float32)

    hT16 = np.ascontiguousarray(hidden_states[0].T).astype(np.float16)

    pos = position_ids[0].astype(np.float64)
    inv_freq = 1.0 / (THETA ** (np.arange(0, D, 2, dtype=np.float64) / D))
    fr = pos[:, None] * inv_freq[None, :]
    c64 = np.cos(fr)
    s64 = np.sin(fr)
    cos128 = np.concatenate([c64, c64], axis=1)
    sin128m = np.concatenate([-s64, s64], axis=1)
    cos5 = np.ascontiguousarray(np.tile(cos128, (1, 5))).astype(np.float16)
    sin5 = np.ascontiguousarray(np.tile(sin128m, (1, 5))).astype(np.float16)

    ii = np.arange(128)
    dmask = (ii[:, None] <= ii[None, :]).astype(np.float16)

    in_maps = []
    for c in range(NCORES):
        wq_c = Wq[c * DOUT:(c + 1) * DOUT].T        # [HID, 512]
        wk_c = Wk[c * D:(c + 1) * D].T              # [HID, 128]
        wv_c = Wv[c * D:(c + 1) * D].T
        wall = np.ascontiguousarray(
            np.concatenate([wq_c, wk_c, wv_c], axis=1)
        ).astype(np.float16)
        wot = np.ascontiguousarray(Wo[:, c * DOUT:(c + 1) * DOUT].T).astype(np.float16)
        in_maps.append({
            "hT": hT16,
            "wall": wall,
            "woT": wot,
            "cos5": cos5,
            "sin5": sin5,
            "dmask": dmask,
        })
    return in_maps


def combine_outputs(results):
    out = np.zeros((S, HID), dtype=np.float32)
    for r in results:
        out += r["outp"]
    k_recent = np.stack([r["krec"] for r in results])[None]  # [1, 8, 208, 128]
    v_recent = np.stack([r["vrec"] for r in results])[None]
    return out[None], k_recent, v_recent


def kernel(hidden_states, position_ids, Wq, Wk, Wv, Wo):
    from concourse.bass_utils import run_bass_kernel_spmd

    nc = get_nc()
    in_maps = make_in_maps(hidden_states, position_ids, Wq, Wk, Wv, Wo)
    res = run_bass_kernel_spmd(nc, in_maps, core_ids=list(range(NCORES)))
    return combine_outputs(res.results)


# revision 29
# speedup vs baseline: 8.3744x; 8.3744x over previous
"""Trainium2 Bass kernel for Llama3-style GQA attention with streaming KV eviction.

Sharding: tensor-parallel over heads across 8 NeuronCores. Each core owns 4
query heads and their single shared KV head (one full GQA group). Wq/Wk/Wv are
sharded on the output (head) dim, Wo on the input dim; the 8 per-core Wo
partials are summed on the host. Sink+recent KV eviction is head-local.

Per-core dataflow (all matmul operands fp16, fp32 PSUM accumulation):
  1. Token-major QKV projection streaming hidden.T in 512-token chunks.
  2. RoPE in token-major layout (rotate-half is a free-dim swap done with a
     single negative-step access pattern), 1/sqrt(D) folded into the exp scale.
  3. PE transposes build d-major qT/kT for attention; v stays token-major.
  4. Scores computed transposed (ST[kt, qt] = kT_blk.T @ qT) so exp(ST) is
     directly the rhs of the P@V matmul - no P transpose, no running max
     (scores are bounded, fp32 exp is safe unnormalized).
  5. Row sums via ones-matmul on PE into one PSUM bank (head h at partition
     32h), reciprocal on DVE, GPSIMD partition-broadcast, normalize into fp16.
  6. Wo matmul from d-major attnT; fp32 partial out DMA'd per core.
"""

import numpy as np

B, S, HID = 1, 2048, 4096
H, KVH, D = 32, 8, 128
THETA = 500000.0
SINK = 4
RECENT = 204  # int(S * 0.1)
NREC = SINK + RECENT  # 208

NCORES = 8
HPC = H // NCORES          # 4 q heads per core
DOUT = HPC * D             # 512
KC = HID // 128            # 32 contraction chunks
TB = S // 128              # 16 token blocks
QC = S // 512              # 4 query chunks
SCALE = float(1.0 / np.sqrt(D))

_CACHE = {}


def _swapped_halves(ap, nchunks):
    """View of ap's first nchunks*128 cols with 64-wide halves swapped per
    128-chunk: out[:, c*128 + j] = ap[:, c*128 + 64 + j] (j<64), ap[:, c*128 + j - 64] (j>=64)."""
    import concourse.bass as bass
    return bass.AP(
        tensor=ap.tensor,
        offset=ap.offset + 64,
        ap=[ap.ap[0], [128, nchunks], [-64, 2], [1, 64]],
    )


def _build_nc():
    import concourse.bass as bass
    import concourse.tile as tile
    from concourse import bacc, mybir
    from concourse.masks import make_identity

    F16 = mybir.dt.float16
    F32 = mybir.dt.float32
    EXP = mybir.ActivationFunctionType.Exp

    nc = bacc.Bacc("TRN2", debug=False)

    hT = nc.dram_tensor("hT", [HID, S], F16, kind="ExternalInput").ap()
    wall = nc.dram_tensor("wall", [HID, DOUT + 256], F16, kind="ExternalInput").ap()
    woT = nc.dram_tensor("woT", [DOUT, HID], F16, kind="ExternalInput").ap()
    cos5 = nc.dram_tensor("cos5", [S, 640], F16, kind="ExternalInput").ap()
    sin5 = nc.dram_tensor("sin5", [S, 640], F16, kind="ExternalInput").ap()
    dmask = nc.dram_tensor("dmask", [128, 128], F16, kind="ExternalInput").ap()
    invd = nc.dram_tensor("invd", [QC, HPC, 512], F32, kind="Internal").ap()
    outp = nc.dram_tensor("outp", [S, HID], F16, kind="ExternalOutput").ap()
    krec = nc.dram_tensor("krec", [NREC, D], F32, kind="ExternalOutput").ap()
    vrec = nc.dram_tensor("vrec", [NREC, D], F32, kind="ExternalOutput").ap()

    with tile.TileContext(nc) as tc:
        from contextlib import ExitStack
        from itertools import cycle

        # round-robin bulk DMAs across the DMA-capable engines' DGE queues so
        # transfers overlap (SP + ACT have HW DGE queues, gpsimd has SWDGE)
        dma_rr = cycle([nc.sync, nc.scalar])

        with ExitStack() as ctx:
            res = ctx.enter_context(tc.tile_pool(name="res", bufs=1))
            qT_sb = res.tile([128, HPC * S], F16)   # [d, h*S + t]
            kT_sb = res.tile([128, S], F16)         # [d, t]
            v_sb = res.tile([128, S], F16)          # [t%128, blk*128 + d]
            attnT = res.tile([128, HPC * S], F16)   # [d, h*S + t]
            ones32 = res.tile([128, 32], F16)
            nc.vector.memset(ones32, 1.0)
            ident = res.tile([128, 128], F16)
            make_identity(nc, ident)
            dmask_sb = res.tile([128, 128], F16)
            nc.sync.dma_start(out=dmask_sb, in_=dmask)
            # wait-carrier: TensorTensor's ISA struct has one sync-wait slot,
            # so pre-consume DMA'd tiles on DVE with tiny copies; later DVE
            # ops then inherit the dep transitively (add_sem_waits elides).
            dm_c = res.tile([1, 2], F16)
            nc.vector.tensor_copy(dm_c, dmask_sb[0:1, 0:2])

            # ---------------- Phase 1: projections + rope + transposes ----
            with ExitStack() as p1:
                wp = p1.enter_context(tc.tile_pool(name="wp", bufs=1))
                ph = p1.enter_context(tc.tile_pool(name="ph", bufs=2))
                trig = p1.enter_context(tc.tile_pool(name="trig", bufs=2))
                rp = p1.enter_context(tc.tile_pool(name="rp", bufs=3))
                pqp = p1.enter_context(tc.tile_pool(name="pqp", bufs=2, space="PSUM"))
                ptr = p1.enter_context(tc.tile_pool(name="ptr", bufs=3, space="PSUM"))

                wall_sb = wp.tile([128, KC, DOUT + 256], F16)
                wall3 = wall.rearrange("(kc p) d -> p kc d", p=128)
                hT3 = hT.rearrange("(kc p) t -> p kc t", p=128)
                cos4 = cos5.rearrange("(n p) d -> p n d", p=128)
                sin4 = sin5.rearrange("(n p) d -> p n d", p=128)

                for tci in range(4):
                    hT_t = ph.tile([128, KC, 512], F16, tag="ht")
                    # 4-kc pieces; for tci 0 interleave the (shared) weight
                    # pieces so the kc=0 matmuls can start after ~1 MB of DMA
                    for g in range(8):
                        if tci == 0:
                            nc.sync.dma_start(
                                out=wall_sb[:, g * 4:(g + 1) * 4, :],
                                in_=wall3[:, g * 4:(g + 1) * 4, :],
                            )
                        nc.sync.dma_start(
                            out=hT_t[:, g * 4:(g + 1) * 4, :],
                            in_=hT3[:, g * 4:(g + 1) * 4, tci * 512:(tci + 1) * 512],
                        )
                    cos_t = trig.tile([128, 4, 640], F16, tag="cos")
                    next(dma_rr).dma_start(out=cos_t, in_=cos4[:, tci * 4:(tci + 1) * 4, :])
                    sin_t = trig.tile([128, 4, 640], F16, tag="sin")
                    next(dma_rr).dma_start(out=sin_t, in_=sin4[:, tci * 4:(tci + 1) * 4, :])
                    tr_c = trig.tile([1, 4], F16, tag="trc")
                    nc.vector.tensor_copy(tr_c[0:1, 0:2], cos_t[0:1, 0, 0:2])
                    nc.vector.tensor_copy(tr_c[0:1, 2:4], sin_t[0:1, 0, 0:2])

                    for tbl in range(4):
                        tb = tci * 4 + tbl
                        pq = pqp.tile([128, DOUT + 256], F32, tag="pq")
                        for kc in range(KC):
                            lhs = hT_t[:, kc, tbl * 128:(tbl + 1) * 128]
                            nc.tensor.matmul(
                                pq[:, 0:512], lhs, wall_sb[:, kc, 0:512],
                                start=(kc == 0), stop=(kc == KC - 1),
                            )
                            nc.tensor.matmul(
                                pq[:, 512:768], lhs, wall_sb[:, kc, 512:768],
                                start=(kc == 0), stop=(kc == KC - 1),
                            )
                        # rope: qk' = qk*cos + swap(qk)*sin_signed
                        rot = rp.tile([128, 640], F16, tag="rot")
                        nc.vector.tensor_copy(rot, _swapped_halves(pq, 5))
                        tmp = rp.tile([128, 640], F16, tag="tmp")
                        nc.vector.tensor_mul(tmp, pq[:, 0:640], cos_t[:, tbl, :])
                        nc.vector.tensor_mul(rot, rot, sin_t[:, tbl, :])
                        qkr = rp.tile([128, 640], F16, tag="qkr")
                        nc.vector.tensor_add(qkr, tmp, rot)
                        # v: straight fp16 copy (token-major)
                        nc.vector.tensor_copy(
                            v_sb[:, tb * 128:(tb + 1) * 128], pq[:, 640:768]
                        )
                        # recents (k roped fp16->fp32 cast; v exact from psum):
                        # cast full aligned tiles, DMA arbitrary row slices.
                        if tb in (0, 14, 15):
                            k32 = rp.tile([128, D], F32, tag="k32")
                            v32 = rp.tile([128, D], F32, tag="v32")
                            nc.vector.tensor_copy(k32, qkr[:, 512:640])
                            nc.vector.tensor_copy(v32, pq[:, 640:768])
                            if tb == 0:
                                nc.sync.dma_start(out=krec[0:4, :], in_=k32[0:4, :])
                                nc.sync.dma_start(out=vrec[0:4, :], in_=v32[0:4, :])
                            elif tb == 14:
                                nc.sync.dma_start(out=krec[4:80, :], in_=k32[52:128, :])
                                nc.sync.dma_start(out=vrec[4:80, :], in_=v32[52:128, :])
                            else:
                                nc.sync.dma_start(out=krec[80:NREC, :], in_=k32)
                                nc.sync.dma_start(out=vrec[80:NREC, :], in_=v32)
                        # transposes to d-major
                        for c in range(5):
                            pt = ptr.tile([128, 128], F16, tag="tr")
                            nc.tensor.transpose(pt, qkr[:, c * 128:(c + 1) * 128], ident)
                            if c < HPC:
                                dst = qT_sb[:, c * S + tb * 128: c * S + (tb + 1) * 128]
                            else:
                                dst = kT_sb[:, tb * 128:(tb + 1) * 128]
                            nc.scalar.copy(dst, pt)

            # Wo weights: prefetch during attention (pool outlives P2)
            wop = ctx.enter_context(tc.tile_pool(name="wop", bufs=1))
            wo_sb = wop.tile([128, HPC, HID], F16)
            wo3 = woT.rearrange("(h p) n -> p h n", p=128)
            for h in range(HPC):
                next(dma_rr).dma_start(out=wo_sb[:, h, :], in_=wo3[:, h, :])

            # ---------------- Phase 2: attention -------------------------
            with ExitStack() as p2:
                ptp = p2.enter_context(tc.tile_pool(name="ptp", bufs=10))
                invp = p2.enter_context(tc.tile_pool(name="invp", bufs=2))
                bbp = p2.enter_context(tc.tile_pool(name="bbp", bufs=3))
                asbp = p2.enter_context(tc.tile_pool(name="asbp", bufs=8))
                pA = p2.enter_context(tc.tile_pool(name="pA", bufs=4, space="PSUM"))
                pT = p2.enter_context(tc.tile_pool(name="pT", bufs=2, space="PSUM"))
                pS = p2.enter_context(tc.tile_pool(name="pS", bufs=2, space="PSUM"))

                for qc in range(QC):
                    nkb = 4 * (qc + 1)
                    As = [pA.tile([128, 512], F32, tag="A", name=f"A{h}") for h in range(HPC)]
                    s_all = pS.tile([128, 512], F32, tag="s")

                    def emit_pv_sum(batch, nkb=nkb, As=As, s_all=s_all, evac=None):
                        for h, j, off, P in batch:
                            first = j == 0
                            last = j == nkb - 1
                            nc.tensor.matmul(
                                As[h][:, off:],
                                v_sb[:, j * 128:(j + 1) * 128],
                                P[:, off:],
                                start=first, stop=last,
                            )
                            nc.tensor.matmul(
                                s_all[32 * h:32 * h + 32, off:],
                                ones32,
                                P[:, off:],
                                start=first, stop=last,
                                tile_position=(0, 32 * h),
                                # sim's psum group-check mis-addresses
                                # partition-offset outputs; data path is fine
                                skip_group_check=True,
                            )
                            if evac is not None:
                                # A[h] complete: evacuate on DVE (ACT must
                                # stay free for the next chunk's exps)
                                nc.vector.tensor_copy(evac[h], As[h])

                    prev = None
                    for j in range(nkb):
                        m = j - 4 * qc
                        off = 128 * m if m >= 0 else 0
                        cur = []
                        for h in range(HPC):
                            T = pT.tile([128, 512], F32, tag="T")
                            nc.tensor.matmul(
                                T[:, off:],
                                kT_sb[:, j * 128:(j + 1) * 128],
                                qT_sb[:, h * S + qc * 512 + off: h * S + (qc + 1) * 512],
                                start=True, stop=True,
                            )
                            P = ptp.tile([128, 512], F16, tag="P")
                            nc.scalar.activation(P[:, off:], T[:, off:], EXP, scale=SCALE)
                            if m >= 0:
                                nc.vector.tensor_mul(
                                    P[:, off:off + 128], P[:, off:off + 128], dmask_sb
                                )
                            cur.append((h, j, off, P))
                        # run the previous kblock's PV/sum while this one's
                        # exps are still in flight (keeps exp latency off PE)
                        if prev is not None:
                            emit_pv_sum(prev)
                        prev = cur
                    emit_pv_sum(prev)
                    inv = invp.tile([128, 512], F32, tag="inv")
                    nc.vector.reciprocal_approx_fast(inv, s_all)
                    a_sb = []
                    for h in range(HPC):
                        a = asbp.tile([128, 512], F32, tag="asb", name=f"asb{h}")
                        nc.vector.tensor_copy(a, As[h])
                        a_sb.append(a)
                    # rows {0,32,64,96} hold the 4 heads' inverses; stage them
                    # in DRAM so a partition-step-0 DMA can broadcast them.
                    inv_rows = bass.AP(
                        tensor=inv.tensor, offset=inv.offset,
                        ap=[[32 * 512, HPC], [1, 512]],
                    )
                    nc.sync.dma_start(out=invd[qc], in_=inv_rows)
                    for h in range(HPC):
                        Bb = bbp.tile([128, 512], F32, tag="B")
                        bcast_src = bass.AP(
                            tensor=invd.tensor,
                            offset=(qc * HPC + h) * 512,
                            ap=[[0, 128], [1, 512]],
                        )
                        nc.sync.dma_start(out=Bb, in_=bcast_src)
                        b_c = bbp.tile([1, 2], F32, tag="bc")
                        nc.vector.tensor_copy(b_c, Bb[0:1, 0:2])
                        nc.vector.tensor_mul(
                            attnT[:, h * S + qc * 512: h * S + (qc + 1) * 512],
                            a_sb[h], Bb,
                        )

            # ---------------- Phase 3: output projection -----------------
            with ExitStack() as p3:
                osbp = p3.enter_context(tc.tile_pool(name="osbp", bufs=6))
                pop = p3.enter_context(tc.tile_pool(name="pop", bufs=8, space="PSUM"))

                for tb in range(TB):
                    pos = [pop.tile([128, 512], F32, tag="po", name=f"po{i}") for i in range(8)]
                    for h in range(HPC):
                        lhs = attnT[:, h * S + tb * 128: h * S + (tb + 1) * 128]
                        for hc in range(8):
                            nc.tensor.matmul(
                                pos[hc], lhs, wo_sb[:, h, hc * 512:(hc + 1) * 512],
                                start=(h == 0), stop=(h == HPC - 1),
                            )
                    for hc in range(8):
                        ot = osbp.tile([128, 512], F16, tag="ot")
                        if hc % 2 == 0:
                            nc.scalar.copy(ot, pos[hc])
                        else:
                            nc.vector.tensor_copy(ot, pos[hc])
                        nc.sync.dma_start(
                            out=outp[tb * 128:(tb + 1) * 128, hc * 512:(hc + 1) * 512],
                            in_=ot,
                        )
    nc.compile()
    return nc


def get_nc():
    if "nc" not in _CACHE:
        _CACHE["nc"] = _build_nc()
    return _CACHE["nc"]


def make_in_maps(hidden_states, position_ids, Wq, Wk, Wv, Wo):
    hidden_states = np.asarray(hidden_states, dtype=np.float32)
    position_ids = np.asarray(position_ids)
    Wq = np.asarray(Wq, dtype=np.float32)
    Wk = np.asarray(Wk, dtype=np.float32)
    Wv = np.asarray(Wv, dtype=np.float32)
    Wo = np.asarray(Wo, dtype=np.float32)

    hT16 = np.ascontiguousarray(hidden_states[0].T).astype(np.float16)

    pos = position_ids[0].astype(np.float64)
    inv_freq = 1.0 / (THETA ** (np.arange(0, D, 2, dtype=np.float64) / D))
    fr = pos[:, None] * inv_freq[None, :]
    c64 = np.cos(fr)
    s64 = np.sin(fr)
    cos128 = np.concatenate([c64, c64], axis=1)
    sin128m = np.concatenate([-s64, s64], axis=1)
    cos5 = np.ascontiguousarray(np.tile(cos128, (1, 5))).astype(np.float16)
    sin5 = np.ascontiguousarray(np.tile(sin128m, (1, 5))).astype(np.float16)

    ii = np.arange(128)
    dmask = (ii[:, None] <= ii[None, :]).astype(np.float16)

    in_maps = []
    for c in range(NCORES):
        wq_c = Wq[c * DOUT:(c + 1) * DOUT].T        # [HID, 512]
        wk_c = Wk[c * D:(c + 1) * D].T              # [HID, 128]
        wv_c = Wv[c * D:(c + 1) * D].T
        wall = np.ascontiguousarray(
            np.concatenate([wq_c, wk_c, wv_c], axis=1)
        ).astype(np.float16)
        wot = np.ascontiguousarray(Wo[:, c * DOUT:(c + 1) * DOUT].T).astype(np.float16)
        in_maps.append({
            "hT": hT16,
            "wall": wall,
            "woT": wot,
            "cos5": cos5,
            "sin5": sin5,
            "dmask": dmask,
        })
    return in_maps


def combine_outputs(results):
    out = np.zeros((S, HID), dtype=np.float32)
    for r in results:
        out += r["outp"]
    k_recent = np.stack([r["krec"] for r in results])[None]  # [1, 8, 208, 128]
    v_recent = np.stack([r["vrec"] for r in results])[None]
    return out[None], k_recent, v_recent


def kernel(hidden_states, position_ids, Wq, Wk, Wv, Wo):
    from concourse.bass_utils import run_bass_kernel_spmd

    nc = get_nc()
    in_maps = make_in_maps(hidden_states, position_ids, Wq, Wk, Wv, Wo)
    res = run_bass_kernel_spmd(nc, in_maps, core_ids=list(range(NCORES)))
    return combine_outputs(res.results)


# revision 32
# speedup vs baseline: 9.9890x; 1.1928x over previous
"""Trainium2 Bass kernel for Llama3-style GQA attention with streaming KV eviction.

Sharding: tensor-parallel over heads across 8 NeuronCores. Each core owns 4
query heads and their single shared KV head (one full GQA group). Wq/Wk/Wv are
sharded on the output (head) dim, Wo on the input dim; the 8 per-core Wo
partials are summed on the host. Sink+recent KV eviction is head-local.

Per-core dataflow (all matmul operands fp16, fp32 PSUM accumulation):
  1. Token-major QKV projection streaming hidden.T in 512-token chunks.
  2. RoPE in token-major layout (rotate-half is a free-dim swap done with a
     single negative-step access pattern), 1/sqrt(D) folded into the exp scale.
  3. PE transposes build d-major qT/kT for attention; v stays token-major.
  4. Scores computed transposed (ST[kt, qt] = kT_blk.T @ qT) so exp(ST) is
     directly the rhs of the P@V matmul - no P transpose, no running max
     (scores are bounded, fp32 exp is safe unnormalized).
  5. Row sums via ones-matmul on PE into one PSUM bank (head h at partition
     32h), reciprocal on DVE, GPSIMD partition-broadcast, normalize into fp16.
  6. Wo matmul from d-major attnT; fp32 partial out DMA'd per core.
"""

import numpy as np

B, S, HID = 1, 2048, 4096
H, KVH, D = 32, 8, 128
THETA = 500000.0
SINK = 4
RECENT = 204  # int(S * 0.1)
NREC = SINK + RECENT  # 208

NCORES = 8
HPC = H // NCORES          # 4 q heads per core
DOUT = HPC * D             # 512
KC = HID // 128            # 32 contraction chunks
TB = S // 128              # 16 token blocks
QC = S // 512              # 4 query chunks
SCALE = float(1.0 / np.sqrt(D))

_CACHE = {}


def _swapped_halves(ap, nchunks):
    """View of ap's first nchunks*128 cols with 64-wide halves swapped per
    128-chunk: out[:, c*128 + j] = ap[:, c*128 + 64 + j] (j<64), ap[:, c*128 + j - 64] (j>=64)."""
    import concourse.bass as bass
    return bass.AP(
        tensor=ap.tensor,
        offset=ap.offset + 64,
        ap=[ap.ap[0], [128, nchunks], [-64, 2], [1, 64]],
    )


def _build_nc():
    import concourse.bass as bass
    import concourse.tile as tile
    from concourse import bacc, mybir
    from concourse.masks import make_identity

    F16 = mybir.dt.float16
    F32 = mybir.dt.float32
    EXP = mybir.ActivationFunctionType.Exp

    nc = bacc.Bacc("TRN2", debug=False)

    hT = nc.dram_tensor("hT", [HID, S], F16, kind="ExternalInput").ap()
    wall = nc.dram_tensor("wall", [HID, DOUT + 256], F16, kind="ExternalInput").ap()
    woT = nc.dram_tensor("woT", [DOUT, HID], F16, kind="ExternalInput").ap()
    cos5 = nc.dram_tensor("cos5", [S, 640], F16, kind="ExternalInput").ap()
    sin5 = nc.dram_tensor("sin5", [S, 640], F16, kind="ExternalInput").ap()
    dmask = nc.dram_tensor("dmask", [128, 128], F16, kind="ExternalInput").ap()
    invd = nc.dram_tensor("invd", [QC, HPC, 512], F32, kind="Internal").ap()
    outp = nc.dram_tensor("outp", [S, HID], F16, kind="ExternalOutput").ap()
    krec = nc.dram_tensor("krec", [NREC, D], F32, kind="ExternalOutput").ap()
    vrec = nc.dram_tensor("vrec", [NREC, D], F32, kind="ExternalOutput").ap()

    with tile.TileContext(nc) as tc:
        from contextlib import ExitStack
        from itertools import cycle

        # round-robin bulk DMAs across the DMA-capable engines' DGE queues so
        # transfers overlap (SP + ACT have HW DGE queues, gpsimd has SWDGE)
        dma_rr = cycle([nc.sync, nc.scalar])

        with ExitStack() as ctx:
            res = ctx.enter_context(tc.tile_pool(name="res", bufs=1))
            qT_sb = res.tile([128, HPC * S], F16)   # [d, h*S + t]
            kT_sb = res.tile([128, S], F16)         # [d, t]
            v_sb = res.tile([128, S], F16)          # [t%128, blk*128 + d]
            attnT = res.tile([128, HPC * S], F16)   # [d, h*S + t]
            ones32 = res.tile([128, 32], F16)
            nc.vector.memset(ones32, 1.0)
            ident = res.tile([128, 128], F16)
            make_identity(nc, ident)
            dmask_sb = res.tile([128, 128], F16)
            nc.sync.dma_start(out=dmask_sb, in_=dmask)
            # wait-carrier: TensorTensor's ISA struct has one sync-wait slot,
            # so pre-consume DMA'd tiles on DVE with tiny copies; later DVE
            # ops then inherit the dep transitively (add_sem_waits elides).
            dm_c = res.tile([1, 2], F16)
            nc.vector.tensor_copy(dm_c, dmask_sb[0:1, 0:2])

            # ---------------- Phase 1: projections + rope + transposes ----
            with ExitStack() as p1:
                wp = p1.enter_context(tc.tile_pool(name="wp", bufs=1))
                ph = p1.enter_context(tc.tile_pool(name="ph", bufs=2))
                trig = p1.enter_context(tc.tile_pool(name="trig", bufs=2))
                rp = p1.enter_context(tc.tile_pool(name="rp", bufs=3))
                pqp = p1.enter_context(tc.tile_pool(name="pqp", bufs=3, space="PSUM"))
                ptr = p1.enter_context(tc.tile_pool(name="ptr", bufs=2, space="PSUM"))

                wall_sb = wp.tile([128, KC, DOUT + 256], F16)
                wall3 = wall.rearrange("(kc p) d -> p kc d", p=128)
                hT3 = hT.rearrange("(kc p) t -> p kc t", p=128)
                cos4 = cos5.rearrange("(n p) d -> p n d", p=128)
                sin4 = sin5.rearrange("(n p) d -> p n d", p=128)

                for tci in range(4):
                    hT_t = ph.tile([128, KC, 512], F16, tag="ht")
                    # kc-split pieces (small first pieces so the kc=0
                    # matmuls start after minimal DMA); for tci 0 interleave
                    # the (shared) weight pieces with the hidden chunks
                    bnds = [0, 2, 4, 8, 12, 16, 20, 24, 28, 32]
                    for lo, hi in zip(bnds[:-1], bnds[1:]):
                        if tci == 0:
                            nc.sync.dma_start(
                                out=wall_sb[:, lo:hi, :],
                                in_=wall3[:, lo:hi, :],
                            )
                        nc.sync.dma_start(
                            out=hT_t[:, lo:hi, :],
                            in_=hT3[:, lo:hi, tci * 512:(tci + 1) * 512],
                        )
                    cos_t = trig.tile([128, 4, 640], F16, tag="cos")
                    next(dma_rr).dma_start(out=cos_t, in_=cos4[:, tci * 4:(tci + 1) * 4, :])
                    sin_t = trig.tile([128, 4, 640], F16, tag="sin")
                    next(dma_rr).dma_start(out=sin_t, in_=sin4[:, tci * 4:(tci + 1) * 4, :])
                    tr_c = trig.tile([1, 4], F16, tag="trc")
                    nc.vector.tensor_copy(tr_c[0:1, 0:2], cos_t[0:1, 0, 0:2])
                    nc.vector.tensor_copy(tr_c[0:1, 2:4], sin_t[0:1, 0, 0:2])

                    for tbl in range(4):
                        tb = tci * 4 + tbl
                        pq = pqp.tile([128, DOUT + 256], F32, tag="pq")
                        for kc in range(KC):
                            lhs = hT_t[:, kc, tbl * 128:(tbl + 1) * 128]
                            nc.tensor.matmul(
                                pq[:, 0:512], lhs, wall_sb[:, kc, 0:512],
                                start=(kc == 0), stop=(kc == KC - 1),
                            )
                            nc.tensor.matmul(
                                pq[:, 512:768], lhs, wall_sb[:, kc, 512:768],
                                start=(kc == 0), stop=(kc == KC - 1),
                            )
                        # rope: qk' = qk*cos + swap(qk)*sin_signed
                        rot = rp.tile([128, 640], F16, tag="rot")
                        nc.vector.tensor_copy(rot, _swapped_halves(pq, 5))
                        tmp = rp.tile([128, 640], F16, tag="tmp")
                        nc.vector.tensor_mul(tmp, pq[:, 0:640], cos_t[:, tbl, :])
                        nc.vector.tensor_mul(rot, rot, sin_t[:, tbl, :])
                        qkr = rp.tile([128, 640], F16, tag="qkr")
                        nc.vector.tensor_add(qkr, tmp, rot)
                        # v: straight fp16 copy (token-major)
                        nc.vector.tensor_copy(
                            v_sb[:, tb * 128:(tb + 1) * 128], pq[:, 640:768]
                        )
                        # recents (k roped fp16->fp32 cast; v exact from psum):
                        # cast full aligned tiles, DMA arbitrary row slices.
                        if tb in (0, 14, 15):
                            k32 = rp.tile([128, D], F32, tag="k32")
                            v32 = rp.tile([128, D], F32, tag="v32")
                            nc.vector.tensor_copy(k32, qkr[:, 512:640])
                            nc.vector.tensor_copy(v32, pq[:, 640:768])
                            if tb == 0:
                                nc.sync.dma_start(out=krec[0:4, :], in_=k32[0:4, :])
                                nc.sync.dma_start(out=vrec[0:4, :], in_=v32[0:4, :])
                            elif tb == 14:
                                nc.sync.dma_start(out=krec[4:80, :], in_=k32[52:128, :])
                                nc.sync.dma_start(out=vrec[4:80, :], in_=v32[52:128, :])
                            else:
                                nc.sync.dma_start(out=krec[80:NREC, :], in_=k32)
                                nc.sync.dma_start(out=vrec[80:NREC, :], in_=v32)
                        # transposes to d-major
                        for c in range(5):
                            pt = ptr.tile([128, 128], F16, tag="tr")
                            nc.tensor.transpose(pt, qkr[:, c * 128:(c + 1) * 128], ident)
                            if c < HPC:
                                dst = qT_sb[:, c * S + tb * 128: c * S + (tb + 1) * 128]
                            else:
                                dst = kT_sb[:, tb * 128:(tb + 1) * 128]
                            nc.scalar.copy(dst, pt)

            # Wo weights: prefetch during attention (pool outlives P2)
            wop = ctx.enter_context(tc.tile_pool(name="wop", bufs=1))
            wo_sb = wop.tile([128, HPC, HID], F16)
            wo3 = woT.rearrange("(h p) n -> p h n", p=128)
            for h in range(HPC):
                next(dma_rr).dma_start(out=wo_sb[:, h, :], in_=wo3[:, h, :])

            # ---------------- Phase 2: attention -------------------------
            with ExitStack() as p2:
                ptp = p2.enter_context(tc.tile_pool(name="ptp", bufs=12))
                invp = p2.enter_context(tc.tile_pool(name="invp", bufs=2))
                bbp = p2.enter_context(tc.tile_pool(name="bbp", bufs=3))
                asbp = p2.enter_context(tc.tile_pool(name="asbp", bufs=8))
                pA = p2.enter_context(tc.tile_pool(name="pA", bufs=4, space="PSUM"))
                pT = p2.enter_context(tc.tile_pool(name="pT", bufs=2, space="PSUM"))
                pS = p2.enter_context(tc.tile_pool(name="pS", bufs=2, space="PSUM"))

                for qc in range(QC):
                    nkb = 4 * (qc + 1)
                    As = [pA.tile([128, 512], F32, tag="A", name=f"A{h}") for h in range(HPC)]
                    s_all = pS.tile([128, 512], F32, tag="s")

                    def emit_pv_sum(batch, nkb=nkb, As=As, s_all=s_all, evac=None):
                        for h, j, off, P in batch:
                            first = j == 0
                            last = j == nkb - 1
                            nc.tensor.matmul(
                                As[h][:, off:],
                                v_sb[:, j * 128:(j + 1) * 128],
                                P[:, off:],
                                start=first, stop=last,
                            )
                            nc.tensor.matmul(
                                s_all[32 * h:32 * h + 32, off:],
                                ones32,
                                P[:, off:],
                                start=first, stop=last,
                                tile_position=(0, 32 * h),
                                # sim's psum group-check mis-addresses
                                # partition-offset outputs; data path is fine
                                skip_group_check=True,
                            )
                            if evac is not None:
                                # A[h] complete: evacuate on DVE (ACT must
                                # stay free for the next chunk's exps)
                                nc.vector.tensor_copy(evac[h], As[h])

                    prev = None
                    for j in range(nkb):
                        m = j - 4 * qc
                        off = 128 * m if m >= 0 else 0
                        cur = []
                        for h in range(HPC):
                            T = pT.tile([128, 512], F32, tag="T")
                            nc.tensor.matmul(
                                T[:, off:],
                                kT_sb[:, j * 128:(j + 1) * 128],
                                qT_sb[:, h * S + qc * 512 + off: h * S + (qc + 1) * 512],
                                start=True, stop=True,
                            )
                            P = ptp.tile([128, 512], F16, tag="P")
                            nc.scalar.activation(P[:, off:], T[:, off:], EXP, scale=SCALE)
                            if m >= 0:
                                nc.vector.tensor_mul(
                                    P[:, off:off + 128], P[:, off:off + 128], dmask_sb
                                )
                            cur.append((h, j, off, P))
                        # run the previous kblock's PV/sum while this one's
                        # exps are still in flight (keeps exp latency off PE)
                        if prev is not None:
                            emit_pv_sum(prev)
                        prev = cur
                    emit_pv_sum(prev)
                    inv = invp.tile([128, 512], F32, tag="inv")
                    nc.vector.reciprocal_approx_fast(inv, s_all)
                    a_sb = []
                    for h in range(HPC):
                        a = asbp.tile([128, 512], F32, tag="asb", name=f"asb{h}")
                        nc.vector.tensor_copy(a, As[h])
                        a_sb.append(a)
                    # rows {0,32,64,96} hold the 4 heads' inverses; stage them
                    # in DRAM so a partition-step-0 DMA can broadcast them.
                    inv_rows = bass.AP(
                        tensor=inv.tensor, offset=inv.offset,
                        ap=[[32 * 512, HPC], [1, 512]],
                    )
                    nc.sync.dma_start(out=invd[qc], in_=inv_rows)
                    for h in range(HPC):
                        Bb = bbp.tile([128, 512], F32, tag="B")
                        bcast_src = bass.AP(
                            tensor=invd.tensor,
                            offset=(qc * HPC + h) * 512,
                            ap=[[0, 128], [1, 512]],
                        )
                        nc.sync.dma_start(out=Bb, in_=bcast_src)
                        b_c = bbp.tile([1, 2], F32, tag="bc")
                        nc.vector.tensor_copy(b_c, Bb[0:1, 0:2])
                        nc.vector.tensor_mul(
                            attnT[:, h * S + qc * 512: h * S + (qc + 1) * 512],
                            a_sb[h], Bb,
                        )

            # ---------------- Phase 3: output projection -----------------
            with ExitStack() as p3:
                osbp = p3.enter_context(tc.tile_pool(name="osbp", bufs=10))
                pop = p3.enter_context(tc.tile_pool(name="pop", bufs=8, space="PSUM"))

                for tb in range(TB):
                    pos = [pop.tile([128, 512], F32, tag="po", name=f"po{i}") for i in range(8)]
                    for h in range(HPC):
                        lhs = attnT[:, h * S + tb * 128: h * S + (tb + 1) * 128]
                        for hc in range(8):
                            nc.tensor.matmul(
                                pos[hc], lhs, wo_sb[:, h, hc * 512:(hc + 1) * 512],
                                start=(h == 0), stop=(h == HPC - 1),
                            )
                    for hc in range(8):
                        ot = osbp.tile([128, 512], F16, tag="ot")
                        if hc % 2 == 0:
                            nc.scalar.copy(ot, pos[hc])
                        else:
                            nc.vector.tensor_copy(ot, pos[hc])
                        nc.sync.dma_start(
                            out=outp[tb * 128:(tb + 1) * 128, hc * 512:(hc + 1) * 512],
                            in_=ot,
                        )
    nc.compile()
    return nc


def get_nc():
    if "nc" not in _CACHE:
        _CACHE["nc"] = _build_nc()
    return _CACHE["nc"]


def make_in_maps(hidden_states, position_ids, Wq, Wk, Wv, Wo):
    hidden_states = np.asarray(hidden_states, dtype=np.float32)
    position_ids = np.asarray(position_ids)
    Wq = np.asarray(Wq, dtype=np.float32)
    Wk = np.asarray(Wk, dtype=np.float32)
    Wv = np.asarray(Wv, dtype=np.float32)
    Wo = np.asarray(Wo, dtype=np.float32)

    hT16 = np.ascontiguousarray(hidden_states[0].T).astype(np.float16)

    pos = position_ids[0].astype(np.float64)
    inv_freq = 1.0 / (THETA ** (np.arange(0, D, 2, dtype=np.float64) / D))
    fr = pos[:, None] * inv_freq[None, :]
    c64 = np.cos(fr)
    s64 = np.sin(fr)
    cos128 = np.concatenate([c64, c64], axis=1)
    sin128m = np.concatenate([-s64, s64], axis=1)
    cos5 = np.ascontiguousarray(np.tile(cos128, (1, 5))).astype(np.float16)
    sin5 = np.ascontiguousarray(np.tile(sin128m, (1, 5))).astype(np.float16)

    ii = np.arange(128)
    dmask = (ii[:, None] <= ii[None, :]).astype(np.float16)

    in_maps = []
    for c in range(NCORES):
        wq_c = Wq[c * DOUT:(c + 1) * DOUT].T        # [HID, 512]
        wk_c = Wk[c * D:(c + 1) * D].T              # [HID, 128]
        wv_c = Wv[c * D:(c + 1) * D].T
        wall = np.ascontiguousarray(
            np.concatenate([wq_c, wk_c, wv_c], axis=1)
        ).astype(np.float16)
        wot = np.ascontiguousarray(Wo[:, c * DOUT:(c + 1) * DOUT].T).astype(np.float16)
        in_maps.append({
            "hT": hT16,
            "wall": wall,
            "woT": wot,
            "cos5": cos5,
            "sin5": sin5,
            "dmask": dmask,
        })
    return in_maps


def combine_outputs(results):
    out = np.zeros((S, HID), dtype=np.float32)
    for r in results:
        out += r["outp"]
    k_recent = np.stack([r["krec"] for r in results])[None]  # [1, 8, 208, 128]
    v_recent = np.stack([r["vrec"] for r in results])[None]
    return out[None], k_recent, v_recent


def kernel(hidden_states, position_ids, Wq, Wk, Wv, Wo):
    from concourse.bass_utils import run_bass_kernel_spmd

    nc = get_nc()
    in_maps = make_in_maps(hidden_states, position_ids, Wq, Wk, Wv, Wo)
    res = run_bass_kernel_spmd(nc, in_maps, core_ids=list(range(NCORES)))
    return combine_outputs(res.results)
